# revision 1
# baseline (speedup 1.0000x reference)
"""PoPE attention Trainium2 kernel, 8-core tensor-parallel (2 heads/core).

Self-contained: hardcodes shapes B=1, S=2048, HID=2048, NH=16, HD=128.

Math (per reference):
  q/k/v = X @ w{q,k,v}.T, split into 16 heads of dim 128
  mu_{q,k} = softplus(q/k)
  q_polar = mu_q * (cos/sin)(pos*invfreq);  k uses angles + clipped bias
  scores  = (q_re.k_re + q_im.k_im)/sqrt(128) + causal_mask
  out     = softmax(scores) @ v;  final = out @ wo.T

Sharding: heads 2c,2c+1 on core c (wq/wk/wv column-sharded). The output
projection is row-sharded after on-device chunked AllGathers of the
per-core attention outputs (one per 512-wide t-chunk, the last chunk
split per head, so communication overlaps attention compute); each core
computes a 256-column slice of the final output (transposed layout),
gathered and transposed on host.

Device layout is feature-major ("transposed"): activations live as
[d, s] so every matmul contracts along partitions with zero on-device
transposes. Scores are computed as E[s', t] so softmax's sum reduction
is a ones-vector matmul and E feeds the A@V matmul directly.
"""

import math
import sys
import types

import numpy as np
import ml_dtypes

import concourse.bass as bass
import concourse.mybir as mybir
import concourse.tile as tile
from concourse.bass_utils import run_bass_kernel_spmd

# ---------------------------------------------------------------- constants
B, S, HID = 1, 2048, 2048
NH, HD = 16, 128
BASE = 10000.0
N_CORES = 8
HPC = NH // N_CORES          # heads per core = 2
DPC = HPC * HD               # head dims per core = 256
P = 128                      # partitions
KO = HID // P                # 16 k-subtiles
NCH = S // 512               # 4 free-dim chunks of 512
SQ = S // P                  # 16 s'-tiles of 128
BF16 = mybir.dt.bfloat16
F32 = mybir.dt.float32
AF = mybir.ActivationFunctionType
ALU = mybir.AluOpType
ISQ = 1.0 / math.sqrt(HD)
NEG = -1.0e9


def _install_ntff_hook():
    """Bare agent image lacks antenv.axon_hooks; synthesize it from the boot
    module's ctypes NTFF hook so run_bass_kernel_spmd(trace=True) works."""
    if "antenv.axon_hooks" in sys.modules:
        return
    try:
        from trn_agent_boot.trn_boot import _ntff_profile_via_ctypes
        hook = _ntff_profile_via_ctypes("/opt/axon/libaxon_pjrt.so")
    except Exception:
        hook = None
    mod = types.ModuleType("antenv.axon_hooks")
    mod.get_axon_ntff_profile_hook = lambda: hook
    mod.set_axon_ntff_profile_hook = lambda h: None
    sys.modules["antenv.axon_hooks"] = mod


_install_ntff_hook()

_TPB_ENGINES = (
    mybir.EngineType.PE,
    mybir.EngineType.Activation,
    mybir.EngineType.DVE,
    mybir.EngineType.Pool,
    mybir.EngineType.SP,
)


class SplitDrainTileContext(tile.TileContext):
    """This walrus build allows at most ONE sem wait per TPB instruction.
    Legalize: move extra waits onto single-wait NOPs emitted just before the
    instruction on the same engine, and split the tail drain the same way."""

    def _split_multiwait(self, insts):
        out = []
        for inst in insts:
            si = getattr(inst, "sync_info", None)
            if (
                si is not None
                and si.on_wait
                and len(si.on_wait) > 1
                and inst.engine in _TPB_ENGINES
            ):
                waits = list(si.on_wait)
                for w in waits[:-1]:
                    out.append(
                        mybir.InstNoOp(
                            name=self.nc.get_next_instruction_name(),
                            sync_info=mybir.SyncInfo(on_wait=[w], on_update=[]),
                            bass_nofuse=True,
                            engine=inst.engine,
                        )
                    )
                si.on_wait = waits[-1:]
            out.append(inst)
        return out

    def _lower_ordered_insts(self, ordered):
        for k in list(ordered.keys()):
            ordered[k] = self._split_multiwait(ordered[k])
        return super()._lower_ordered_insts(ordered)

    def _drain_and_barrier(self, tick_clock, wait_clock):
        from concourse.vector_clock import ScopedClock

        drain_inst = self.nc.sync.drain()
        wait_clock.add_sem_waits(
            drain_inst.ins, ScopedClock({None: tick_clock.global_clock})
        )
        waits = list(drain_inst.ins.sync_info.on_wait or [])
        if len(waits) > 1:
            drain_inst.ins.sync_info.on_wait = waits[:1]
            for w in waits[1:]:
                d2 = self.nc.sync.drain()
                if d2.ins.sync_info is None:
                    d2.ins.sync_info = mybir.SyncInfo(on_wait=[w], on_update=[])
                else:
                    d2.ins.sync_info.on_wait = [w]

        self.nc.all_engine_barrier()
        assert self.sems is not None
        popped = self.nc._tile_sem_poison_stack.pop()
        assert popped is self._sem_poison
        self.nc.clear_and_free_semaphores(list(self.sems.allocated().values()))
        self.nc.all_engine_barrier()


def build_nc():
    nc = bass.Bass("TRN2", target_bir_lowering=False, debug=False,
                   num_devices=N_CORES)

    xt_d = nc.dram_tensor("xt", [HID, S], BF16, kind="ExternalInput").ap()
    wq_d = nc.dram_tensor("wq", [HID, DPC], BF16, kind="ExternalInput").ap()
    wk_d = nc.dram_tensor("wk", [HID, DPC], BF16, kind="ExternalInput").ap()
    wv_d = nc.dram_tensor("wv", [HID, DPC], BF16, kind="ExternalInput").ap()
    # wo rows permuted so even-head dims come first (matches the
    # head-split ag-half layout used by the output projection)
    wo2_d = nc.dram_tensor("wo2", [HID, DPC], BF16, kind="ExternalInput").ap()
    cos_d = nc.dram_tensor("cosT", [P, S], BF16, kind="ExternalInput").ap()
    sin_d = nc.dram_tensor("sinT", [P, S], BF16, kind="ExternalInput").ap()
    cb_d = nc.dram_tensor("cb", [P, HPC], F32, kind="ExternalInput").ap()
    sb_d = nc.dram_tensor("sb", [P, HPC], F32, kind="ExternalInput").ap()
    tri_d = nc.dram_tensor("tri", [P, P], F32, kind="ExternalInput").ap()
    out_d = nc.dram_tensor("out", [DPC, S], F32, kind="ExternalOutput").ap()

    with SplitDrainTileContext(nc) as tc:
        with tc.tile_pool(name="big", bufs=1) as big, \
             tc.tile_pool(name="wts", bufs=1) as wts, \
             tc.tile_pool(name="tabs", bufs=1) as tabs, \
             tc.tile_pool(name="qk", bufs=2) as qkp, \
             tc.tile_pool(name="mu", bufs=3) as mup, \
             tc.tile_pool(name="tmp", bufs=4) as tmpp, \
             tc.tile_pool(name="ep", bufs=6) as ep, \
             tc.tile_pool(name="sm", bufs=2) as smp, \
             tc.tile_pool(name="ob", bufs=2) as obp, \
             tc.tile_pool(name="ag", bufs=2) as agp, \
             tc.tile_pool(name="ps", bufs=1, space="PSUM") as psp, \
             tc.tile_pool(name="dram", bufs=1, space="DRAM") as dram:

            # ---------------- loads -----------------------------------
            # xt arrives in four 512-wide column pieces (all ko per piece)
            # so projections for s-chunk n can start after piece n lands.
            # Order: everything piece 0 needs first.
            wq_sb = wts.tile([P, KO, DPC], BF16, name="wq_sb")
            nc.sync.dma_start(wq_sb[:], wq_d.rearrange("(ko p) o -> p ko o", p=P))

            xt_sb = big.tile([P, KO, S], BF16, tag="big", name="xt_sb")
            xr = xt_d.rearrange("(ko p) s -> p ko s", p=P)
            def load_piece(n):
                ch = slice(512 * n, 512 * (n + 1))
                for ko in range(KO):
                    nc.sync.dma_start(xt_sb[:, ko, ch], xr[:, ko, ch])

            load_piece(0)

            wk_sb = wts.tile([P, KO, DPC], BF16, name="wk_sb")
            nc.sync.dma_start(wk_sb[:], wk_d.rearrange("(ko p) o -> p ko o", p=P))
            cos_sb = tabs.tile([P, S], BF16, name="cos_sb")
            nc.sync.dma_start(cos_sb[:], cos_d[:])
            sin_sb = tabs.tile([P, S], BF16, name="sin_sb")
            nc.sync.dma_start(sin_sb[:], sin_d[:])
            cb_sb = tabs.tile([P, HPC], F32, name="cb_sb")
            nc.sync.dma_start(cb_sb[:], cb_d[:])
            sb_sb = tabs.tile([P, HPC], F32, name="sb_sb")
            nc.sync.dma_start(sb_sb[:], sb_d[:])
            tri_sb = tabs.tile([P, P], F32, name="tri_sb")
            nc.sync.dma_start(tri_sb[:], tri_d[:])
            wv_sb = wts.tile([P, KO, DPC], BF16, name="wv_sb")
            nc.sync.dma_start(wv_sb[:], wv_d.rearrange("(ko p) o -> p ko o", p=P))

            for n in range(1, NCH):
                load_piece(n)

            ones_k = tabs.tile([P, 1], BF16, name="ones_k")
            nc.gpsimd.memset(ones_k[:], 1.0)
            ones_m = tabs.tile([1, P], F32, name="ones_m")
            nc.gpsimd.memset(ones_m[:], 1.0)

            # AllGather bounce buffers. Chunks 0..2 gather both heads at
            # once; chunk 3 is split per head so its first AG fires a head
            # earlier and the tail shrinks.
            cc_in = [dram.tile([DPC, 512], BF16, name=f"cc_in{j}")
                     for j in range(NCH - 1)]
            cc_out = [dram.tile([NH * HD, 512], BF16, addr_space="Shared",
                                name=f"cc_out{j}") for j in range(NCH - 1)]
            cc3_in = [dram.tile([P, 512], BF16, name=f"cc3_in{h}")
                      for h in range(HPC)]
            ccw_in = dram.tile([1, 64], BF16, name="ccw_in")
            ccw_out = dram.tile([N_CORES, 64], BF16, addr_space="Shared",
                                name="ccw_out")
            cc3_out = [dram.tile([N_CORES * P, 512], BF16,
                                 addr_space="Shared", name=f"cc3_out{h}")
                       for h in range(HPC)]

            # ---------------- QKV projections (s-chunk major) ---------
            q_re = {}
            q_im = {}
            k_re = {}
            k_im = {}
            for h in range(HPC):
                q_re[h] = qkp.tile([P, S], BF16, tag="q_re", name=f"q_re{h}")
                q_im[h] = qkp.tile([P, S], BF16, tag="q_im", name=f"q_im{h}")
                k_re[h] = qkp.tile([P, S], BF16, tag="k_re", name=f"k_re{h}")
                k_im[h] = qkp.tile([P, S], BF16, tag="k_im", name=f"k_im{h}")

            for n in range(NCH):
                ch = slice(512 * n, 512 * (n + 1))
                for h in range(HPC):
                    hsl = slice(P * h, P * (h + 1))
                    # --- Q
                    pq = psp.tile([P, 512], F32, tag="pp", bufs=2, name="pq")
                    for ko in range(KO):
                        nc.tensor.matmul(pq[:], wq_sb[:, ko, hsl],
                                         xt_sb[:, ko, ch],
                                         start=(ko == 0), stop=(ko == KO - 1))
                    # softplus(x) = ln(exp(x) + 1); Softplus has no ACT table
                    # set in this build, Exp/Ln share one.
                    eq = mup.tile([P, 512], F32, tag="mu", name="eq")
                    nc.scalar.activation(eq[:], pq[:], AF.Exp)
                    mu = mup.tile([P, 512], F32, tag="mu", name="mu_q")
                    nc.scalar.activation(mu[:], eq[:], AF.Ln, bias=1.0)
                    nc.vector.tensor_tensor(q_re[h][:, ch], mu[:],
                                            cos_sb[:, ch], ALU.mult)
                    nc.vector.tensor_tensor(q_im[h][:, ch], mu[:],
                                            sin_sb[:, ch], ALU.mult)
                    # --- K
                    pk = psp.tile([P, 512], F32, tag="pp", bufs=2, name="pk")
                    for ko in range(KO):
                        nc.tensor.matmul(pk[:], wk_sb[:, ko, hsl],
                                         xt_sb[:, ko, ch],
                                         start=(ko == 0), stop=(ko == KO - 1))
                    ek = mup.tile([P, 512], F32, tag="mu", name="ek")
                    nc.scalar.activation(ek[:], pk[:], AF.Exp)
                    muk = mup.tile([P, 512], F32, tag="mu", name="mu_k")
                    nc.scalar.activation(muk[:], ek[:], AF.Ln, bias=1.0)
                    # k_re = (muk*cb)*cos - (muk*sb)*sin
                    # k_im = (muk*sb)*cos + (muk*cb)*sin
                    ta = tmpp.tile([P, 512], F32, tag="tmp", name="ta")
                    nc.vector.scalar_tensor_tensor(
                        ta[:], muk[:], cb_sb[:, h:h + 1], cos_sb[:, ch],
                        ALU.mult, ALU.mult)
                    tb = tmpp.tile([P, 512], F32, tag="tmp", name="tb")
                    nc.vector.scalar_tensor_tensor(
                        tb[:], muk[:], sb_sb[:, h:h + 1], sin_sb[:, ch],
                        ALU.mult, ALU.mult)
                    nc.vector.tensor_tensor(k_re[h][:, ch], ta[:], tb[:],
                                            ALU.subtract)
                    tcn = tmpp.tile([P, 512], F32, tag="tmp", name="tc")
                    nc.vector.scalar_tensor_tensor(
                        tcn[:], muk[:], sb_sb[:, h:h + 1], cos_sb[:, ch],
                        ALU.mult, ALU.mult)
                    td = tmpp.tile([P, 512], F32, tag="tmp", name="td")
                    nc.vector.scalar_tensor_tensor(
                        td[:], muk[:], cb_sb[:, h:h + 1], sin_sb[:, ch],
                        ALU.mult, ALU.mult)
                    nc.vector.tensor_tensor(k_im[h][:, ch], tcn[:], td[:],
                                            ALU.add)
                # --- V for the four s'-tiles inside this piece
                for i in range(4 * n, 4 * n + 4):
                    ssl = slice(P * i, P * (i + 1))
                    pv = psp.tile([P, DPC], F32, tag="pp", bufs=2, name="pv")
                    for ko in range(KO):
                        nc.tensor.matmul(pv[:], xt_sb[:, ko, ssl],
                                         wv_sb[:, ko, :],
                                         start=(ko == 0), stop=(ko == KO - 1))
                    if n == 0 and i == 0:
                        v_sb = big.tile([P, SQ, DPC], BF16, tag="vsb",
                                        name="v_sb")
                    nc.vector.tensor_copy(out=v_sb[:, i, :], in_=pv[:])

            # Warm up the collectives stream early (ring setup costs ~20us
            # on the first op); a 512-byte gather during QKV absorbs it.
            nc.gpsimd.dma_start(ccw_in[:], ones_k[:64, 0:1].rearrange('p o -> o p'))
            nc.gpsimd.collective_compute(
                "AllGather", ALU.bypass,
                replica_groups=[list(range(N_CORES))],
                ins=[ccw_in[:]], outs=[ccw_out[:]],
            )

            # wo weights: emitted now; they finish long before needed.
            wo2_sb = wts.tile([P, KO, DPC], BF16, name="wo2_sb")
            nc.sync.dma_start(wo2_sb[:], wo2_d.rearrange("(ko p) o -> p ko o", p=P))

            # ---------------- attention, t-chunk major ----------------
            # The per-(head,chunk) normalize/broadcast tail (pb/bc/osb/DMA)
            # is deferred until the next block's first scores are in flight,
            # so the PE never head-of-line waits on the ACT recip chain.
            deferred = []

            for j in range(NCH):
                for h in range(HPC):
                    hsl = slice(P * h, P * (h + 1))
                    nlive = 4 * j + 4
                    pav = psp.tile([P, 512], F32, tag="pav", bufs=2,
                                   name="pav")
                    psum1 = psp.tile([1, 512], F32, tag="prs", bufs=1,
                                     name="psum1")
                    # software-pipelined (depth 2): emit rowsum/AV for
                    # iteration i-2 after iteration i's exp, so the PE never
                    # head-of-line blocks on an exp that isn't done yet.
                    pend = []

                    def emit_rs_av(e_t, i_t, nlive=nlive, pav=pav,
                                   psum1=psum1, hsl=hsl):
                        nc.tensor.matmul(psum1[:], ones_k[:], e_t[:],
                                         start=(i_t == 0),
                                         stop=(i_t == nlive - 1))
                        nc.tensor.matmul(pav[:], v_sb[:, i_t, hsl], e_t[:],
                                         start=(i_t == 0),
                                         stop=(i_t == nlive - 1))

                    for i in range(nlive):
                        r = i - 4 * j
                        t0 = 0 if r < 0 else P * r
                        tvs = slice(512 * j + t0, 512 * (j + 1))
                        lvs = slice(t0, 512)
                        ps = psp.tile([P, 512], F32, tag="ps", bufs=3,
                                      name="ps")
                        ksl = slice(P * i, P * (i + 1))
                        nc.tensor.matmul(ps[:, lvs], k_re[h][:, ksl],
                                         q_re[h][:, tvs],
                                         start=True, stop=False)
                        nc.tensor.matmul(ps[:, lvs], k_im[h][:, ksl],
                                         q_im[h][:, tvs],
                                         start=False, stop=True)
                        if r >= 0:
                            # diagonal 128-col sub-block gets causal mask
                            nc.vector.tensor_tensor(
                                ps[:, t0:t0 + P], ps[:, t0:t0 + P],
                                tri_sb[:], ALU.add)
                        e = ep.tile([P, 512], BF16, tag="e", name="e")
                        if r > 0:
                            nc.gpsimd.memset(e[:, 0:t0], 0.0)
                        nc.scalar.activation(e[:, lvs], ps[:, lvs], AF.Exp,
                                             scale=ISQ)
                        pend.append((e, i))
                        if len(pend) > 2:
                            emit_rs_av(*pend.pop(0))
                        if i == 1:
                            for fin in deferred:
                                fin()
                            deferred = []
                    for p_ in pend:
                        emit_rs_av(*p_)
                    # rec = 1/rowsum via exp(-ln(x)): same ACT table set as
                    # the attention exps. Emitted now so psum1 frees early;
                    # the PE-side tail is deferred.
                    lnt = smp.tile([1, 512], F32, tag="lnt", name="lnt")
                    nc.scalar.activation(lnt[:], psum1[:], AF.Ln)
                    rec = smp.tile([1, 512], F32, tag="rec", name="rec")
                    nc.scalar.activation(rec[:], lnt[:], AF.Exp, scale=-1.0)

                    def finalize(rec=rec, pav=pav, h=h, j=j):
                        pb = psp.tile([P, 512], F32, tag="pp", bufs=2,
                                      name="pb")
                        nc.tensor.matmul(pb[:], ones_m[:], rec[:],
                                         start=True, stop=True)
                        bc = smp.tile([P, 512], F32, tag="bc", name="bc")
                        nc.vector.tensor_copy(out=bc[:], in_=pb[:])
                        osb = obp.tile([P, 512], BF16, tag="osb", name="osb")
                        nc.vector.tensor_tensor(osb[:], pav[:], bc[:],
                                                ALU.mult)
                        if j < NCH - 1:
                            nc.gpsimd.dma_start(cc_in[j][P * h:P * (h + 1), :],
                                                osb[:])
                        else:
                            nc.gpsimd.dma_start(cc3_in[h][:, :], osb[:])
                            nc.gpsimd.collective_compute(
                                "AllGather", ALU.bypass,
                                replica_groups=[list(range(N_CORES))],
                                ins=[cc3_in[h][:]], outs=[cc3_out[h][:]],
                            )

                    deferred.append(finalize)
                if j < NCH - 1:
                    def chunk_ag(j=j):
                        nc.gpsimd.collective_compute(
                            "AllGather", ALU.bypass,
                            replica_groups=[list(range(N_CORES))],
                            ins=[cc_in[j][:]], outs=[cc_out[j][:]],
                        )
                    deferred.append(chunk_ag)
            for fin in deferred:
                fin()
            deferred = []

            # ---------------- output projection -----------------------
            # ag buffers are half-chunks [P, 8, 512] in a 3-deep pool so the
            # next chunk's gather-load overlaps this chunk's matmuls.
            for j in range(NCH):
                ch = slice(512 * j, 512 * (j + 1))
                quarters = []
                for qi in range(4):
                    agt = agp.tile([P, KO // 4, 512], BF16, tag="ag",
                                   name=f"ag{j}_{qi}")
                    for kl in range(KO // 4):
                        c = 4 * (qi % 2) + kl
                        if j < NCH - 1:
                            src = cc_out[j][256 * c + P * (qi // 2):
                                            256 * c + P * (qi // 2) + P, :]
                        else:
                            src = cc3_out[qi // 2][P * c:P * c + P, :]
                        nc.sync.dma_start(agt[:, kl, :], src)
                    quarters.append(agt)
                w_sb = wo2_sb
                for m in range(HPC):
                    msl = slice(P * m, P * (m + 1))
                    po = psp.tile([P, 512], F32, tag="pp", bufs=2, name="po")
                    for ko in range(KO):
                        nc.tensor.matmul(po[:], w_sb[:, ko, msl],
                                         quarters[ko // (KO // 4)]
                                         [:, ko % (KO // 4), :],
                                         start=(ko == 0), stop=(ko == KO - 1))
                    fo = obp.tile([P, 512], F32, tag="fo", name="fo")
                    nc.vector.tensor_copy(out=fo[:], in_=po[:])
                    nc.sync.dma_start(out_d[msl, ch], fo[:])

    return nc


_NC_CACHE = None
_LAST_IN_MAPS = None


def _get_nc():
    global _NC_CACHE
    if _NC_CACHE is None:
        _NC_CACHE = build_nc()
    return _NC_CACHE


def kernel(hidden_states, wq, wk, wv, wo, learned_bias, attention_mask):
    bf16 = ml_dtypes.bfloat16
    x = np.asarray(hidden_states, dtype=np.float32).reshape(S, HID)
    xt = np.ascontiguousarray(x.T).astype(bf16)

    wqT = np.asarray(wq, dtype=np.float32).T.astype(bf16)   # [HID, out]
    wkT = np.asarray(wk, dtype=np.float32).T.astype(bf16)
    wvT = np.asarray(wv, dtype=np.float32).T.astype(bf16)
    woT = np.asarray(wo, dtype=np.float32).T.astype(bf16)

    # row permutation matching the head-split AllGather of the last chunk:
    # even heads' dims (rank-major) first, then odd heads'.
    perm = np.concatenate([
        np.arange(NH * HD).reshape(NH, HD)[h::HPC].reshape(-1)
        for h in range(HPC)
    ])
    woT2 = np.ascontiguousarray(woT[perm])

    inv_freq = 1.0 / (BASE ** (np.arange(HD, dtype=np.float32) / HD))
    pos = np.arange(S, dtype=np.float32)
    freqs = pos[:, None] * inv_freq[None, :]                # [S, HD]
    cosT = np.ascontiguousarray(np.cos(freqs).T).astype(bf16)  # [HD, S]
    sinT = np.ascontiguousarray(np.sin(freqs).T).astype(bf16)

    bias = np.clip(np.asarray(learned_bias, dtype=np.float32),
                   -2.0 * math.pi, 0.0).reshape(NH, HD)     # [NH, HD]
    cbias = np.cos(bias).astype(np.float32)
    sbias = np.sin(bias).astype(np.float32)

    tri = np.where(np.arange(P)[:, None] > np.arange(P)[None, :],
                   np.float32(NEG), np.float32(0.0)).astype(np.float32)

    in_maps = []
    for c in range(N_CORES):
        osl = slice(DPC * c, DPC * (c + 1))
        heads = slice(HPC * c, HPC * (c + 1))
        in_maps.append({
            "xt": xt,
            "wq": np.ascontiguousarray(wqT[:, osl]),
            "wk": np.ascontiguousarray(wkT[:, osl]),
            "wv": np.ascontiguousarray(wvT[:, osl]),
            "wo2": np.ascontiguousarray(woT2[:, osl]),
            "cosT": cosT,
            "sinT": sinT,
            "cb": np.ascontiguousarray(cbias[heads].T),     # [HD, HPC]
            "sb": np.ascontiguousarray(sbias[heads].T),
            "tri": tri,
        })

    global _LAST_IN_MAPS
    _LAST_IN_MAPS = in_maps
    nc = _get_nc()
    res = run_bass_kernel_spmd(nc, in_maps, list(range(N_CORES)))
    finalT = np.concatenate([res.results[c]["out"] for c in range(N_CORES)],
                            axis=0)                          # [HID, S]
    return np.ascontiguousarray(finalT.T)[None].astype(np.float32)



# revision 3
# speedup vs baseline: 1.0401x; 1.0401x over previous
"""PoPE attention Trainium2 kernel, 8-core tensor-parallel (2 heads/core).

Self-contained: hardcodes shapes B=1, S=2048, HID=2048, NH=16, HD=128.

Math (per reference):
  q/k/v = X @ w{q,k,v}.T, split into 16 heads of dim 128
  mu_{q,k} = softplus(q/k)
  q_polar = mu_q * (cos/sin)(pos*invfreq);  k uses angles + clipped bias
  scores  = (q_re.k_re + q_im.k_im)/sqrt(128) + causal_mask
  out     = softmax(scores) @ v;  final = out @ wo.T

Sharding: heads 2c,2c+1 on core c (wq/wk/wv column-sharded). The output
projection is row-sharded after on-device chunked AllGathers of the
per-core attention outputs; each core computes a 256-column slice of the
final output (transposed layout), gathered and transposed on host.

Device layout is feature-major ("transposed"): activations live as
[d, s] so every matmul contracts along partitions with zero on-device
transposes. Scores are computed as E[s', t] so softmax's sum reduction
is a ones-vector matmul and E feeds the A@V matmul directly.

The k-side rotation bias is folded into per-head cos/sin tables on the
host, so k_re/k_im are single multiplies. The whole kernel is emitted as
one software-pipelined stream: projections for chunk j+1 and the output
projection for chunk j-1 are interleaved between the attention blocks of
chunk j, so the PE never drains while waiting on activations or
AllGathers. The causal mask is a multiplicative 0/1 triangle applied to
E after the exp, and diagonal-tile AV/rowsum matmuls run on partial
free ranges instead of zero-padding.
"""

import math
import sys
import types

import numpy as np
import ml_dtypes

import concourse.bass as bass
import concourse.mybir as mybir
import concourse.tile as tile
from concourse.bass_utils import run_bass_kernel_spmd

# ---------------------------------------------------------------- constants
B, S, HID = 1, 2048, 2048
NH, HD = 16, 128
BASE = 10000.0
N_CORES = 8
HPC = NH // N_CORES          # heads per core = 2
DPC = HPC * HD               # head dims per core = 256
P = 128                      # partitions
KO = HID // P                # 16 k-subtiles
NCH = S // 512               # 4 free-dim chunks of 512
SQ = S // P                  # 16 s'-tiles of 128
BF16 = mybir.dt.bfloat16
F32 = mybir.dt.float32
AF = mybir.ActivationFunctionType
ALU = mybir.AluOpType
ISQ = 1.0 / math.sqrt(HD)


def _install_ntff_hook():
    """Bare agent image lacks antenv.axon_hooks; synthesize it from the boot
    module's ctypes NTFF hook so run_bass_kernel_spmd(trace=True) works."""
    if "antenv.axon_hooks" in sys.modules:
        return
    try:
        from trn_agent_boot.trn_boot import _ntff_profile_via_ctypes
        hook = _ntff_profile_via_ctypes("/opt/axon/libaxon_pjrt.so")
    except Exception:
        hook = None
    mod = types.ModuleType("antenv.axon_hooks")
    mod.get_axon_ntff_profile_hook = lambda: hook
    mod.set_axon_ntff_profile_hook = lambda h: None
    sys.modules["antenv.axon_hooks"] = mod


_install_ntff_hook()

_TPB_ENGINES = (
    mybir.EngineType.PE,
    mybir.EngineType.Activation,
    mybir.EngineType.DVE,
    mybir.EngineType.Pool,
    mybir.EngineType.SP,
)


class SplitDrainTileContext(tile.TileContext):
    """This walrus build allows at most ONE sem wait per TPB instruction.
    Legalize: move extra waits onto single-wait NOPs emitted just before the
    instruction on the same engine, and split the tail drain the same way."""

    def _split_multiwait(self, insts):
        out = []
        for inst in insts:
            si = getattr(inst, "sync_info", None)
            if (
                si is not None
                and si.on_wait
                and len(si.on_wait) > 1
                and inst.engine in _TPB_ENGINES
            ):
                waits = list(si.on_wait)
                for w in waits[:-1]:
                    out.append(
                        mybir.InstNoOp(
                            name=self.nc.get_next_instruction_name(),
                            sync_info=mybir.SyncInfo(on_wait=[w], on_update=[]),
                            bass_nofuse=True,
                            engine=inst.engine,
                        )
                    )
                si.on_wait = waits[-1:]
            out.append(inst)
        return out

    def _lower_ordered_insts(self, ordered):
        for k in list(ordered.keys()):
            ordered[k] = self._split_multiwait(ordered[k])
        return super()._lower_ordered_insts(ordered)

    def _drain_and_barrier(self, tick_clock, wait_clock):
        from concourse.vector_clock import ScopedClock

        drain_inst = self.nc.sync.drain()
        wait_clock.add_sem_waits(
            drain_inst.ins, ScopedClock({None: tick_clock.global_clock})
        )
        waits = list(drain_inst.ins.sync_info.on_wait or [])
        if len(waits) > 1:
            drain_inst.ins.sync_info.on_wait = waits[:1]
            for w in waits[1:]:
                d2 = self.nc.sync.drain()
                if d2.ins.sync_info is None:
                    d2.ins.sync_info = mybir.SyncInfo(on_wait=[w], on_update=[])
                else:
                    d2.ins.sync_info.on_wait = [w]

        self.nc.all_engine_barrier()
        assert self.sems is not None
        popped = self.nc._tile_sem_poison_stack.pop()
        assert popped is self._sem_poison
        self.nc.clear_and_free_semaphores(list(self.sems.allocated().values()))
        self.nc.all_engine_barrier()


def build_nc():
    nc = bass.Bass("TRN2", target_bir_lowering=False, debug=False,
                   num_devices=N_CORES)

    xt_d = nc.dram_tensor("xt", [HID, S], BF16, kind="ExternalInput").ap()
    wq_d = nc.dram_tensor("wq", [HID, DPC], BF16, kind="ExternalInput").ap()
    wk_d = nc.dram_tensor("wk", [HID, DPC], BF16, kind="ExternalInput").ap()
    wv_d = nc.dram_tensor("wv", [HID, DPC], BF16, kind="ExternalInput").ap()
    # wo rows permuted so even-head dims come first (matches the
    # head-split ag-half layout used by the output projection)
    wo2_d = nc.dram_tensor("wo2", [HID, DPC], BF16, kind="ExternalInput").ap()
    cos_d = nc.dram_tensor("cosT", [P, S], BF16, kind="ExternalInput").ap()
    sin_d = nc.dram_tensor("sinT", [P, S], BF16, kind="ExternalInput").ap()
    # per-head k-side tables with the clipped bias folded in: [P, HPC, S]
    ck_d = nc.dram_tensor("ckT", [P, HPC, S], BF16, kind="ExternalInput").ap()
    sk_d = nc.dram_tensor("skT", [P, HPC, S], BF16, kind="ExternalInput").ap()
    tri_d = nc.dram_tensor("tri", [P, P], BF16, kind="ExternalInput").ap()
    out_d = nc.dram_tensor("out", [DPC, S], BF16, kind="ExternalOutput").ap()

    with SplitDrainTileContext(nc) as tc:
        with tc.tile_pool(name="big", bufs=1) as big, \
             tc.tile_pool(name="wts", bufs=1) as wts, \
             tc.tile_pool(name="tabs", bufs=1) as tabs, \
             tc.tile_pool(name="qk", bufs=2) as qkp, \
             tc.tile_pool(name="mu", bufs=4) as mup, \
             tc.tile_pool(name="ep", bufs=6) as ep, \
             tc.tile_pool(name="sm", bufs=2) as smp, \
             tc.tile_pool(name="ob", bufs=2) as obp, \
             tc.tile_pool(name="ag", bufs=2) as agp, \
             tc.tile_pool(name="ps", bufs=1, space="PSUM") as psp, \
             tc.tile_pool(name="dram", bufs=1, space="DRAM") as dram:

            # ---------------- loads (just-in-time order) ----------------
            wq_sb = wts.tile([P, KO, DPC], BF16, name="wq_sb")
            nc.sync.dma_start(wq_sb[:], wq_d.rearrange("(ko p) o -> p ko o", p=P))

            xt_sb = big.tile([P, KO, S], BF16, tag="big", name="xt_sb")
            xr = xt_d.rearrange("(ko p) s -> p ko s", p=P)
            # piece 0 split per-ko so Q(0) matmuls can chase the DMAs
            for ko in range(KO):
                nc.sync.dma_start(xt_sb[:, ko, 0:512], xr[:, ko, 0:512])

            wk_sb = wts.tile([P, KO, DPC], BF16, name="wk_sb")
            nc.sync.dma_start(wk_sb[:], wk_d.rearrange("(ko p) o -> p ko o", p=P))

            # chunk-0 columns of the four rotation tables, rest later
            cos_sb = tabs.tile([P, S], BF16, name="cos_sb")
            sin_sb = tabs.tile([P, S], BF16, name="sin_sb")
            ck_sb = tabs.tile([P, HPC, S], BF16, name="ck_sb")
            sk_sb = tabs.tile([P, HPC, S], BF16, name="sk_sb")
            nc.sync.dma_start(cos_sb[:, 0:512], cos_d[:, 0:512])
            nc.sync.dma_start(sin_sb[:, 0:512], sin_d[:, 0:512])
            nc.sync.dma_start(ck_sb[:, :, 0:512], ck_d[:, :, 0:512])
            nc.sync.dma_start(sk_sb[:, :, 0:512], sk_d[:, :, 0:512])
            tri_sb = tabs.tile([P, P], BF16, name="tri_sb")
            nc.sync.dma_start(tri_sb[:], tri_d[:])

            wv_sb = wts.tile([P, KO, DPC], BF16, name="wv_sb")
            nc.sync.dma_start(wv_sb[:], wv_d.rearrange("(ko p) o -> p ko o", p=P))

            nc.sync.dma_start(xt_sb[:, :, 512:1024], xr[:, :, 512:1024])
            nc.sync.dma_start(cos_sb[:, 512:], cos_d[:, 512:])
            nc.sync.dma_start(sin_sb[:, 512:], sin_d[:, 512:])
            nc.sync.dma_start(ck_sb[:, :, 512:], ck_d[:, :, 512:])
            nc.sync.dma_start(sk_sb[:, :, 512:], sk_d[:, :, 512:])
            nc.sync.dma_start(xt_sb[:, :, 1024:1536], xr[:, :, 1024:1536])
            nc.sync.dma_start(xt_sb[:, :, 1536:2048], xr[:, :, 1536:2048])

            wo2_sb = wts.tile([P, KO, DPC], BF16, name="wo2_sb")
            nc.sync.dma_start(wo2_sb[:], wo2_d.rearrange("(ko p) o -> p ko o", p=P))

            ones_k = tabs.tile([P, 1], BF16, name="ones_k")
            nc.gpsimd.memset(ones_k[:], 1.0)
            ones_m = tabs.tile([1, P], BF16, name="ones_m")
            nc.gpsimd.memset(ones_m[:], 1.0)

            # AllGather bounce buffers. Chunks 0..2 gather both heads at
            # once; chunk 3 is split per head so its first AG fires a head
            # earlier and the tail shrinks.
            cc_in = [dram.tile([DPC, 512], BF16, name=f"cc_in{j}")
                     for j in range(NCH - 1)]
            cc_out = [dram.tile([NH * HD, 512], BF16, addr_space="Shared",
                                name=f"cc_out{j}") for j in range(NCH - 1)]
            cc3_in = [dram.tile([P, 512], BF16, name=f"cc3_in{h}")
                      for h in range(HPC)]
            ccw_in = dram.tile([1, 64], BF16, name="ccw_in")
            ccw_out = dram.tile([N_CORES, 64], BF16, addr_space="Shared",
                                name="ccw_out")
            cc3_out = [dram.tile([N_CORES * P, 512], BF16,
                                 addr_space="Shared", name=f"cc3_out{h}")
                       for h in range(HPC)]

            # persistent activation tiles
            q_re = {}
            q_im = {}
            k_re = {}
            k_im = {}
            for h in range(HPC):
                q_re[h] = qkp.tile([P, S], BF16, tag="q_re", name=f"q_re{h}")
                q_im[h] = qkp.tile([P, S], BF16, tag="q_im", name=f"q_im{h}")
                k_re[h] = qkp.tile([P, S], BF16, tag="k_re", name=f"k_re{h}")
                k_im[h] = qkp.tile([P, S], BF16, tag="k_im", name=f"k_im{h}")
            v_sb = big.tile([P, SQ, DPC], BF16, tag="vsb", name="v_sb")

            # ---------------- emission units ---------------------------
            def proj_q(n, h):
                ch = slice(512 * n, 512 * (n + 1))
                hsl = slice(P * h, P * (h + 1))
                pq = psp.tile([P, 512], F32, tag="pp", bufs=2, name="pq")
                for ko in range(KO):
                    nc.tensor.matmul(pq[:], wq_sb[:, ko, hsl],
                                     xt_sb[:, ko, ch],
                                     start=(ko == 0), stop=(ko == KO - 1))
                # softplus(x) = ln(exp(x) + 1); Softplus has no ACT table
                # set in this build, Exp/Ln share one.
                eq = mup.tile([P, 512], BF16, tag="mu", name="eq")
                nc.scalar.activation(eq[:], pq[:], AF.Exp)
                mu = mup.tile([P, 512], BF16, tag="mu", name="mu_q")
                nc.scalar.activation(mu[:], eq[:], AF.Ln, bias=1.0)
                nc.vector.tensor_tensor(q_re[h][:, ch], mu[:],
                                        cos_sb[:, ch], ALU.mult)
                nc.vector.tensor_tensor(q_im[h][:, ch], mu[:],
                                        sin_sb[:, ch], ALU.mult)

            def proj_k(n, h):
                ch = slice(512 * n, 512 * (n + 1))
                hsl = slice(P * h, P * (h + 1))
                pk = psp.tile([P, 512], F32, tag="pp", bufs=2, name="pk")
                for ko in range(KO):
                    nc.tensor.matmul(pk[:], wk_sb[:, ko, hsl],
                                     xt_sb[:, ko, ch],
                                     start=(ko == 0), stop=(ko == KO - 1))
                ek = mup.tile([P, 512], BF16, tag="mu", name="ek")
                nc.scalar.activation(ek[:], pk[:], AF.Exp)
                muk = mup.tile([P, 512], BF16, tag="mu", name="mu_k")
                nc.scalar.activation(muk[:], ek[:], AF.Ln, bias=1.0)
                nc.vector.tensor_tensor(k_re[h][:, ch], muk[:],
                                        ck_sb[:, h, ch], ALU.mult)
                nc.vector.tensor_tensor(k_im[h][:, ch], muk[:],
                                        sk_sb[:, h, ch], ALU.mult)

            def proj_v(i):
                # V for s'-tile i, both heads; [s', d] layout for A@V
                ssl = slice(P * i, P * (i + 1))
                pv = psp.tile([P, DPC], F32, tag="pp", bufs=2, name="pv")
                for ko in range(KO):
                    nc.tensor.matmul(pv[:], xt_sb[:, ko, ssl],
                                     wv_sb[:, ko, :],
                                     start=(ko == 0), stop=(ko == KO - 1))
                nc.vector.tensor_copy(out=v_sb[:, i, :], in_=pv[:])

            def attn_block(j, h):
                """Scores/exp/rowsum/AV + normalize + gather-DMA for one
                (t-chunk, head). Diagonal s'-tiles first (r=0 covers the
                full free range and opens the PSUM accumulation), then the
                full sub-diagonal tiles; the last emitted closes it."""
                hsl = slice(P * h, P * (h + 1))
                nlive = 4 * j + 4
                if j == 0:
                    order = list(range(4))
                else:
                    order = list(range(4 * j, 4 * j + 4)) + list(range(4 * j))
                pav = psp.tile([P, 512], F32, tag="pav", bufs=2, name="pav")
                psum1 = psp.tile([1, 512], F32, tag="prs", bufs=1,
                                 name="psum1")
                pend = []

                def emit_rs_av(e_t, lvs_t, i_t, first_t, last_t):
                    nc.tensor.matmul(psum1[0:1, lvs_t], ones_k[:],
                                     e_t[:, lvs_t],
                                     start=first_t, stop=last_t)
                    nc.tensor.matmul(pav[:, lvs_t], v_sb[:, i_t, hsl],
                                     e_t[:, lvs_t],
                                     start=first_t, stop=last_t)

                for idx, i in enumerate(order):
                    r = i - 4 * j
                    t0 = 0 if r <= 0 else P * r
                    tvs = slice(512 * j + t0, 512 * (j + 1))
                    evs = slice(t0, 512)          # exp'd (valid) range
                    # chunk 0 keeps full-F accumulation (zero-padded e);
                    # later chunks accumulate the partial range only
                    lvs = slice(0, 512) if j == 0 else evs
                    ps = psp.tile([P, 512], F32, tag="ps", bufs=3, name="ps")
                    ksl = slice(P * i, P * (i + 1))
                    nc.tensor.matmul(ps[:, evs], k_re[h][:, ksl],
                                     q_re[h][:, tvs],
                                     start=True, stop=False)
                    nc.tensor.matmul(ps[:, evs], k_im[h][:, ksl],
                                     q_im[h][:, tvs],
                                     start=False, stop=True)
                    e = ep.tile([P, 512], BF16, tag="e", name="e")
                    if j == 0 and r > 0:
                        nc.gpsimd.memset(e[:, 0:t0], 0.0)
                    nc.scalar.activation(e[:, evs], ps[:, evs], AF.Exp,
                                         scale=ISQ)
                    if r >= 0:
                        # multiplicative 0/1 causal triangle on the
                        # diagonal 128-col sub-block
                        nc.vector.tensor_tensor(e[:, t0:t0 + P],
                                                e[:, t0:t0 + P],
                                                tri_sb[:], ALU.mult)
                    pend.append((e, lvs, i, idx == 0, idx == nlive - 1))
                    if len(pend) > 2:
                        emit_rs_av(*pend.pop(0))
                for et in pend:
                    emit_rs_av(*et)

                # rec = 1/rowsum via exp(-ln(x)): same ACT table set as
                # the attention exps.
                lnt = smp.tile([1, 512], F32, tag="lnt", name="lnt")
                nc.scalar.activation(lnt[:], psum1[:], AF.Ln)
                rec = smp.tile([1, 512], BF16, tag="rec", name="rec")
                nc.scalar.activation(rec[:], lnt[:], AF.Exp, scale=-1.0)
                pb = psp.tile([P, 512], F32, tag="pp", bufs=2, name="pb")
                nc.tensor.matmul(pb[:], ones_m[:], rec[:],
                                 start=True, stop=True)
                bc = smp.tile([P, 512], BF16, tag="bc", name="bc")
                nc.vector.tensor_copy(out=bc[:], in_=pb[:])
                osb = obp.tile([P, 512], BF16, tag="osb", name="osb")
                nc.vector.tensor_tensor(osb[:], pav[:], bc[:], ALU.mult)
                if j < NCH - 1:
                    nc.gpsimd.dma_start(cc_in[j][P * h:P * (h + 1), :],
                                        osb[:])
                else:
                    nc.gpsimd.dma_start(cc3_in[h][:, :], osb[:])
                    nc.gpsimd.collective_compute(
                        "AllGather", ALU.bypass,
                        replica_groups=[list(range(N_CORES))],
                        ins=[cc3_in[h][:]], outs=[cc3_out[h][:]],
                    )

            def chunk_ag(j):
                nc.gpsimd.collective_compute(
                    "AllGather", ALU.bypass,
                    replica_groups=[list(range(N_CORES))],
                    ins=[cc_in[j][:]], outs=[cc_out[j][:]],
                )

            def outproj(j):
                ch = slice(512 * j, 512 * (j + 1))
                quarters = []
                for qi in range(4):
                    agt = agp.tile([P, KO // 4, 512], BF16, tag="ag",
                                   name=f"ag{j}_{qi}")
                    for kl in range(KO // 4):
                        c = 4 * (qi % 2) + kl
                        if j < NCH - 1:
                            src = cc_out[j][256 * c + P * (qi // 2):
                                            256 * c + P * (qi // 2) + P, :]
                        else:
                            src = cc3_out[qi // 2][P * c:P * c + P, :]
                        nc.sync.dma_start(agt[:, kl, :], src)
                    quarters.append(agt)
                for m in range(HPC):
                    msl = slice(P * m, P * (m + 1))
                    po = psp.tile([P, 512], F32, tag="pp", bufs=2, name="po")
                    for ko in range(KO):
                        nc.tensor.matmul(po[:], wo2_sb[:, ko, msl],
                                         quarters[ko // (KO // 4)]
                                         [:, ko % (KO // 4), :],
                                         start=(ko == 0), stop=(ko == KO - 1))
                    fo = obp.tile([P, 512], BF16, tag="fo", name="fo")
                    nc.vector.tensor_copy(out=fo[:], in_=po[:])
                    nc.sync.dma_start(out_d[msl, ch], fo[:])

            # ---------------- pipelined emission -----------------------
            proj_q(0, 0)
            proj_k(0, 0)
            proj_q(0, 1)
            proj_k(0, 1)
            for i in range(4):
                proj_v(i)

            # Warm up the collectives stream early (ring setup costs ~20us
            # on the first op); a tiny gather during attn(0) absorbs it.
            nc.gpsimd.dma_start(ccw_in[:],
                                ones_k[:64, 0:1].rearrange('p o -> o p'))
            nc.gpsimd.collective_compute(
                "AllGather", ALU.bypass,
                replica_groups=[list(range(N_CORES))],
                ins=[ccw_in[:]], outs=[ccw_out[:]],
            )

            for j in range(NCH):
                attn_block(j, 0)
                if j < NCH - 1:
                    proj_q(j + 1, 0)
                    proj_k(j + 1, 0)
                attn_block(j, 1)
                if j < NCH - 1:
                    proj_q(j + 1, 1)
                    proj_k(j + 1, 1)
                    for i in range(4 * (j + 1), 4 * (j + 1) + 4):
                        proj_v(i)
                    chunk_ag(j)
                if j >= 1:
                    outproj(j - 1)
            outproj(NCH - 1)

    return nc


_NC_CACHE = None
_LAST_IN_MAPS = None


def _get_nc():
    global _NC_CACHE
    if _NC_CACHE is None:
        _NC_CACHE = build_nc()
    return _NC_CACHE


def kernel(hidden_states, wq, wk, wv, wo, learned_bias, attention_mask):
    bf16 = ml_dtypes.bfloat16
    x = np.asarray(hidden_states, dtype=np.float32).reshape(S, HID)
    xt = np.ascontiguousarray(x.T).astype(bf16)

    wqT = np.asarray(wq, dtype=np.float32).T.astype(bf16)   # [HID, out]
    wkT = np.asarray(wk, dtype=np.float32).T.astype(bf16)
    wvT = np.asarray(wv, dtype=np.float32).T.astype(bf16)
    woT = np.asarray(wo, dtype=np.float32).T.astype(bf16)

    # row permutation matching the head-split AllGather of the last chunk:
    # even heads' dims (rank-major) first, then odd heads'.
    perm = np.concatenate([
        np.arange(NH * HD).reshape(NH, HD)[h::HPC].reshape(-1)
        for h in range(HPC)
    ])
    woT2 = np.ascontiguousarray(woT[perm])

    inv_freq = 1.0 / (BASE ** (np.arange(HD, dtype=np.float32) / HD))
    pos = np.arange(S, dtype=np.float32)
    freqs = pos[:, None] * inv_freq[None, :]                # [S, HD]
    cosT = np.ascontiguousarray(np.cos(freqs).T).astype(bf16)  # [HD, S]
    sinT = np.ascontiguousarray(np.sin(freqs).T).astype(bf16)

    bias = np.clip(np.asarray(learned_bias, dtype=np.float32),
                   -2.0 * math.pi, 0.0).reshape(NH, HD)     # [NH, HD]
    # k-side tables with bias folded: [HD, NH, S]
    kf = freqs[None, :, :] + bias[:, None, :]               # [NH, S, HD]
    ckT = np.ascontiguousarray(np.cos(kf).transpose(2, 0, 1)).astype(bf16)
    skT = np.ascontiguousarray(np.sin(kf).transpose(2, 0, 1)).astype(bf16)

    tri = (np.arange(P)[:, None] <= np.arange(P)[None, :]).astype(bf16)

    in_maps = []
    for c in range(N_CORES):
        osl = slice(DPC * c, DPC * (c + 1))
        heads = slice(HPC * c, HPC * (c + 1))
        in_maps.append({
            "xt": xt,
            "wq": np.ascontiguousarray(wqT[:, osl]),
            "wk": np.ascontiguousarray(wkT[:, osl]),
            "wv": np.ascontiguousarray(wvT[:, osl]),
            "wo2": np.ascontiguousarray(woT2[:, osl]),
            "cosT": cosT,
            "sinT": sinT,
            "ckT": np.ascontiguousarray(ckT[:, heads, :]),
            "skT": np.ascontiguousarray(skT[:, heads, :]),
            "tri": tri,
        })

    global _LAST_IN_MAPS
    _LAST_IN_MAPS = in_maps
    nc = _get_nc()
    res = run_bass_kernel_spmd(nc, in_maps, list(range(N_CORES)))
    finalT = np.concatenate([np.asarray(res.results[c]["out"],
                                        dtype=np.float32)
                             for c in range(N_CORES)], axis=0)  # [HID, S]
    return np.ascontiguousarray(finalT.T)[None].astype(np.float32)
